# revision 13
# baseline (speedup 1.0000x reference)
"""NT-Xent loss kernel for Trainium2 (8 NeuronCores, SPMD).

Math: for z (2N, D), rows L2-normalized, sim = z_n @ z_n.T / T with T=0.5.
loss = mean_i( logsumexp_{j != i}(sim[i, j]) - sim[i, pos(i)] ).
Since sim entries are bounded in [-2, 2], logsumexp needs no max pass:
lse_i = log( sum_j exp(sim[i, j]) - exp(sim[i, i]) ), and sim[i, i] = 2
(self cosine == 1), so the diagonal mask is a constant e^2 subtraction.

Sharding: core c receives z rolled by -1024*c rows, so every core's block
is rows 0:1024 and positive partners are rows 4096:5120 of its own input
(column order of the candidate set does not affect the lse). Each core
computes its 1024x8192 sim block with PE matmuls in bf16, does the
exp-row-sum on ScalarE straight out of PSUM, and returns 16 partial sums
(8 columns of per-m-tile lse sums, 8 columns of per-row-tile pos-dot
sums). Host reduces 8x16 numbers to the scalar loss.

Engine layout per core:
- GPSIMD/SWDGE: fp32->bf16 cast-loads of z row tiles (keeps the HWDGE
  ring free for transposes).
- DVE: bn_stats row stats -> sumsq, normalize scale (tensor_scalar), pos
  dots. No custom-ISA DVE ops (tensor_tensor_reduce/reciprocal hang HW
  on this PJRT path).
- ScalarE: rsqrt via exp(-0.5*ln(s)) (same ACT table set as the main
  exp), the exp row-sum accumulation out of PSUM, final log.
- Sync/HWDGE: one DMA-transpose per row tile into zT.
- PE: 512 accumulating 128x128x512 bf16 matmuls + ones-vector partition
  reduction.
"""

import sys

sys.path.insert(0, "/opt/trn_rl_repo")

import numpy as np

import concourse.bass as bass
import concourse.bacc as bacc
import concourse.mybir as mybir
from concourse.tile import TileContext

TWO_N = 8192
D = 512
NCORES = 8
B = TWO_N // NCORES  # 1024 rows per core
P = 128
RT = TWO_N // P  # 64 row tiles
BT = B // P  # 8 block m-tiles per core
KT = D // P  # 4 contraction chunks
NW = 512  # matmul moving free dim
NT = TWO_N // NW  # 16 n-tiles
NG = 4  # n-tiles per PSUM group (4 banks)
POS_T = (TWO_N // 2) // P  # row-tile offset of positive partners (32)
E2 = float(np.exp(2.0))
_DEBUG_PHASES = "ABC"  # internal: limit traced phases when bisecting

f32 = mybir.dt.float32
bf16 = mybir.dt.bfloat16
ALU = mybir.AluOpType
ACTF = mybir.ActivationFunctionType


def build_tile_body(nc, z_ap, out_ap):
    """Trace the whole SPMD per-core program under a TileContext."""
    with TileContext(nc) as tc:
        with (
            tc.tile_pool(name="sb", bufs=1) as sb,
            tc.tile_pool(name="ps", bufs=2, space="PSUM") as psp,
        ):
            # Persistent strips
            bnst = sb.tile([P, RT * 6], f32, tag="bnst", name="bnst")
            bn3 = bnst.rearrange("p (r s) -> p r s", s=6)
            sumsq = sb.tile([P, RT], f32, tag="sumsq", name="sumsq")
            lnn = sb.tile([P, RT], f32, tag="lnn", name="lnn")
            invn = sb.tile([P, RT], f32, tag="invn", name="invn")
            sgrp = sb.tile([P, BT * NG], f32, tag="sgrp", name="sgrp")
            s8 = sb.tile([P, BT], f32, tag="s8", name="s8")
            # vstrip: cols 0:8 = lse per m-tile, cols 8:16 = pos dot per row-tile
            vstrip = sb.tile([P, 2 * BT], f32, tag="vstrip", name="vstrip")
            ones = sb.tile([P, 1], f32, tag="ones", name="ones")
            nege2 = sb.tile([P, 1], f32, tag="nege2", name="nege2")
            outsb = sb.tile([1, 2 * BT], f32, tag="outsb", name="outsb")

            nc.vector.memset(ones, 1.0)
            nc.vector.memset(nege2, -E2)

            # zT in bf16, layout [q, r, k, p] so each row-tile transpose
            # writes one contiguous 1KB/partition range (clean subtile deps)
            zt = sb.tile([P, RT * KT * P], bf16, tag="zt", name="zt")
            zt4 = zt.rearrange("q (r k p) -> q r k p", k=KT, p=P)

            # ---------------- Phase A: cast-load, stats, normalize, transpose
            zraw = {}
            z16s = {}
            for r in range(RT):
                zr = sb.tile([P, D], bf16, tag="z16r", bufs=10, name=f"zr_{r}")
                # SWDGE cast-DMA: fp32 DRAM -> bf16 SBUF
                nc.gpsimd.dma_start(out=zr, in_=z_ap[r * P : (r + 1) * P, :])
                # per-row stats: (cnt, mean, n*var) for even/odd element halves
                nc.vector.bn_stats(bn3[:, r, :], zr)
                zraw[r] = zr
                if r % 8 == 7:
                    g8 = r // 8
                    lo, hi = g8 * 8, (g8 + 1) * 8
                    me = bn3[:, lo:hi, 1]
                    ve = bn3[:, lo:hi, 2]
                    mo = bn3[:, lo:hi, 4]
                    vo = bn3[:, lo:hi, 5]
                    # sumsq = ve + vo + 256*(me^2 + mo^2)
                    ta = sb.tile([P, 8], f32, tag="bns_a", bufs=2, name=f"ta_{g8}")
                    tb = sb.tile([P, 8], f32, tag="bns_b", bufs=2, name=f"tb_{g8}")
                    nc.vector.tensor_mul(ta, me, me)
                    nc.vector.tensor_mul(tb, mo, mo)
                    nc.vector.tensor_add(ta, ta, tb)
                    nc.vector.tensor_scalar_mul(ta, ta, float(D // 2))
                    nc.vector.tensor_add(ta, ta, ve)
                    nc.vector.tensor_add(sumsq[:, lo:hi], ta, vo)
                    # 1/sqrt(s) = exp(-0.5 * ln(s)) — same ACT table set as Exp
                    nc.scalar.activation(lnn[:, lo:hi], sumsq[:, lo:hi], ACTF.Ln)
                    nc.scalar.activation(
                        invn[:, lo:hi], lnn[:, lo:hi], ACTF.Exp, scale=-0.5
                    )
                    for rr in range(lo, hi):
                        keep = rr < BT or POS_T <= rr < POS_T + BT
                        if keep:
                            zn = sb.tile(
                                [P, D], bf16, tag=f"z16n_{rr}", name=f"zn_{rr}"
                            )
                        else:
                            zn = sb.tile(
                                [P, D], bf16, tag="z16n", bufs=4, name=f"zn_{rr}"
                            )
                        nc.vector.tensor_scalar_mul(
                            zn, zraw[rr], invn[:, rr : rr + 1]
                        )
                        if keep:
                            z16s[rr] = zn
                        # one transpose per row tile: (128, 512) -> [:, rr, :, :]
                        nc.sync.dma_start_transpose(
                            out=zt4[:, rr, :, :], in_=zn
                        )
                        del zraw[rr]

            # ---------------- Phase B: Gram block matmuls + exp row sums
            if "B" not in _DEBUG_PHASES:
                nc.vector.memset(sgrp, 1.0)
            for g in range(NT // NG if "B" in _DEBUG_PHASES else 0):
                for m in range(BT):
                    psum = psp.tile([P, NG * NW], f32, tag="ps", name=f"ps_{g}_{m}")
                    for k in range(KT):
                        lhsT = zt4[:, m, k, :]
                        for j in range(NG):
                            n = g * NG + j
                            nc.tensor.matmul(
                                psum[:, j * NW : (j + 1) * NW],
                                lhsT,
                                zt4[:, 4 * n : 4 * n + 4, k, :],
                                start=(k == 0),
                                stop=(k == KT - 1),
                            )
                    ex = sb.tile([P, NG * NW], bf16, tag="ex", bufs=2, name=f"ex_{g}_{m}")
                    nc.scalar.activation(
                        ex,
                        psum,
                        ACTF.Exp,
                        scale=2.0,
                        accum_out=sgrp[:, m * NG + g : m * NG + g + 1],
                    )

            # ---------------- Phase C: reduce + log + pos + partition sum
            nc.vector.tensor_reduce(
                out=s8,
                in_=sgrp.rearrange("p (m g) -> p m g", g=NG),
                axis=mybir.AxisListType.X,
                op=ALU.add,
            )
            # lse per row = log(S - e^2)
            nc.scalar.activation(vstrip[:, 0:BT], s8, ACTF.Ln, bias=nege2)
            for t in range(BT):
                psq = sb.tile([P, D], f32, tag="sq", bufs=2, name=f"psq_{t}")
                nc.vector.tensor_mul(psq, z16s[t], z16s[POS_T + t])
                nc.vector.tensor_reduce(
                    out=vstrip[:, BT + t : BT + t + 1],
                    in_=psq,
                    axis=mybir.AxisListType.X,
                    op=ALU.add,
                )
            pso = psp.tile([1, 2 * BT], f32, tag="ps", name="pso")
            nc.tensor.matmul(pso, ones, vstrip, start=True, stop=True)
            nc.scalar.copy(outsb, pso)
            nc.gpsimd.dma_start(out=out_ap, in_=outsb)


_NC_CACHE = {}


def _build_nc():
    if "nc" not in _NC_CACHE:
        nc = bacc.Bacc("TRN2")
        z = nc.dram_tensor("z", (TWO_N, D), f32, kind="ExternalInput")
        out = nc.dram_tensor("out", (1, 2 * BT), f32, kind="ExternalOutput")
        build_tile_body(nc, z.ap(), out.ap())
        nc.compile()
        _NC_CACHE["nc"] = nc
    return _NC_CACHE["nc"]


def make_in_maps(z):
    z32 = np.ascontiguousarray(z, dtype=np.float32)
    return [
        {"z": np.ascontiguousarray(np.roll(z32, -B * c, axis=0))}
        for c in range(NCORES)
    ]


def combine(results):
    total = 0.0
    for c in range(NCORES):
        v = np.asarray(results[c]["out"], dtype=np.float64).reshape(-1)
        total += v[:BT].sum() - 2.0 * v[BT:].sum()
    return np.float32(total / TWO_N)


def kernel(z, _spmd_kwargs=None):
    from concourse import bass_utils

    nc = _build_nc()
    res = bass_utils.run_bass_kernel_spmd(
        nc,
        make_in_maps(z),
        core_ids=list(range(NCORES)),
        **(_spmd_kwargs or {}),
    )
    out = combine(res.results)
    if _spmd_kwargs:
        return out, res
    return out
